# revision 9
# baseline (speedup 1.0000x reference)
"""Trainium2 Bass kernel for nn_Decode (gnn_message_passing).

Full inputs in, full output out. Internally: data-parallel over the edge
dimension E=2048 across 8 NeuronCores (256 edges each). Per core:

  gather src/relation embeddings (indirect DMA) -> DRAM re-layout ->
  conv1d(k=3, SAME) as PE matmuls (slab-packed M=100 = (c, d-half)) ->
  training-mode BN1 stats on ACT/DVE + AllReduce -> affine+relu ->
  fc [E,10000]@[10000,200]^T as per-d K=100 fp32r matmuls accumulating in
  PSUM -> BN2 stats + AllReduce -> affine+relu -> cosine similarities via
  PE dot-reductions -> scatter c[e] into out[e, o[e]] (output pre-zeroed).

BatchNorm batch statistics span all 2048 edges -> two tiny AllReduces.
Conv/fc biases cancel inside BatchNorm and are ignored.
"""
import numpy as np

NC = 8
E_FULL = 2048
ESH = E_FULL // NC        # 256 edges per core
EH = 128                  # e-half (gather granularity)
D = 200
C = 50
NN = 40943
R = 474
EPS_BN = 1e-5
EPS_COS = 1e-8
G = 10                    # dl-group size for fc streaming
NDQ = 100 // G
N1 = float(E_FULL * D)    # BN1 stat count
N2 = float(E_FULL)        # BN2 stat count

# window table: (k, sl) -> (dl_lo, dl_hi, src_lo); src col = dl + 100*sl + k - 1
WIN = {
    (0, 0): (1, 100, 0),
    (1, 0): (0, 100, 0),
    (2, 0): (0, 100, 1),
    (0, 1): (0, 100, 99),
    (1, 1): (0, 100, 100),
    (2, 1): (0, 99, 101),
}
# xs slabs: (base_partition, n_ci, E_w, tensor-map per (ci, epart))
# tensors: 0=s 1=p1 2=et 3=p2a 4=p2b 5=p3a 6=p3b 7=p3c
SLABS = [
    ("b2", 0, 2, 256, {(0, 0): 0, (0, 1): 0, (1, 0): 1, (1, 1): 2}),
    ("b3", 32, 3, 128, {(0, 0): 0, (1, 0): 3, (2, 0): 4}),
    ("b4", 64, 4, 128, {(0, 0): 0, (1, 0): 5, (2, 0): 6, (3, 0): 7}),
]
# blk order in fc/z columns: 0=x1 1=x2 2=x3 3=re

_CACHE = {}


def _build(debug=False):
    import concourse.bass as bass
    import concourse.bacc as bacc
    import concourse.tile as tile
    import concourse.mybir as mybir

    dt = mybir.dt
    AF = mybir.ActivationFunctionType
    OP = mybir.AluOpType

    nc = bacc.Bacc("TRN2", target_bir_lowering=False, debug=False, num_devices=NC)

    def din(name, shape, dtype=dt.float32):
        return nc.dram_tensor(name, shape, dtype, kind="ExternalInput").ap()

    t_ptab = din("ptab", [ESH, D])
    t_remb = din("remb", [R, D])
    t_idx = din("idx_all", [16, EH], dt.int32)
    t_w2f = din("w2f", [12, 100])
    t_w3f = din("w3f", [18, 100])
    t_w4f = din("w4f", [24, 100])
    t_fcw = din("fcw", [100, 100, D])
    t_pair = din("pairmat", [100, 100])
    t_g1 = din("g1be1", [100, 2])
    t_g2 = din("g2be2", [D, 2])
    t_sidx = din("scatidx", [2, EH], dt.int32)
    t_out = nc.dram_tensor("out", [ESH, NN], dt.float32, kind="ExternalOutput").ap()
    dbg = {}
    if debug:
        for nm, shp in (("d_stats", [100, 8]), ("d_gstats", [100, 8]),
                        ("d_af1", [100, 8]), ("d_u2", [100, 5120]),
                        ("d_z", [128, 1024]), ("d_st2a", [128, 8]),
                        ("d_af2a", [128, 8]), ("d_ds", [128, 8]),
                        ("d_xs", [88, 25600]), ("d_zb", [72, 1024]),
                        ("d_xna", [128, 1024]), ("d_xnb", [72, 1024]),
                        ("d_csb", [128, 2])):
            dbg[nm] = nc.dram_tensor(nm, shp, dt.float32, kind="ExternalOutput").ap()

    with tile.TileContext(nc) as tc:
        with (
            tc.tile_pool(name="p1", bufs=1) as p1,
            tc.tile_pool(name="p2", bufs=2) as p2,
            tc.tile_pool(name="p3", bufs=3) as p3,
            tc.tile_pool(name="pc", bufs=2, space="PSUM") as pc,
            tc.tile_pool(name="pz", bufs=1, space="PSUM") as pz,
            tc.tile_pool(name="dr", bufs=1, space="DRAM") as dr,
        ):
            # ---------- persistent SBUF ----------
            xs = p1.tile([88, 256 * 100], dt.float32r, tag="xs")
            w2s = p1.tile([12, 100], dt.float32r, tag="w2s")
            w3s = p1.tile([50, 100], dt.float32r, tag="w3s")
            w4s = p1.tile([88, 100], dt.float32r, tag="w4s")
            pair_sb = p1.tile([100, 100], dt.float32, tag="pair")
            g1_sb = p1.tile([100, 2], dt.float32, tag="g1")
            g2a_sb = p1.tile([128, 2], dt.float32, tag="g2a")
            g2b_sb = p1.tile([72, 2], dt.float32, tag="g2b")
            sqacc = p1.tile([100, 256], dt.float32, tag="sqacc")
            ssum = p1.tile([100, 256], dt.float32, tag="ssum")
            stats = p1.tile([100, 8], dt.float32, tag="stats")
            gstats = p1.tile([100, 8], dt.float32, tag="gstats")
            fs_sb = p1.tile([100, 8], dt.float32, tag="fs")
            af1 = p1.tile([100, 8], dt.float32, tag="af1")
            st2a = p1.tile([128, 8], dt.float32, tag="st2a")
            st2b = p1.tile([72, 8], dt.float32, tag="st2b")
            gst2a = p1.tile([128, 8], dt.float32, tag="gst2a")
            gst2b = p1.tile([72, 8], dt.float32, tag="gst2b")
            af2a = p1.tile([128, 8], dt.float32, tag="af2a")
            af2b = p1.tile([72, 8], dt.float32, tag="af2b")
            xna = p1.tile([128, 1024], dt.float32, tag="xna")
            xnb = p1.tile([72, 1024], dt.float32, tag="xnb")
            ones = p1.tile([128, 1], dt.float32, tag="ones")
            zcol = p1.tile([EH, 1], dt.float32, tag="zcol")

            nc.gpsimd.memset(ones[:], 1.0)
            nc.gpsimd.memset(zcol[:], 0.0)

            # weights / constants in
            nc.gpsimd.dma_start(out=w2s[:], in_=t_w2f[:])
            nc.gpsimd.dma_start(out=w3s[32:50], in_=t_w3f[:])
            nc.gpsimd.dma_start(out=w4s[64:88], in_=t_w4f[:])
            nc.sync.dma_start(out=pair_sb[:], in_=t_pair[:])
            nc.sync.dma_start(out=g1_sb[:], in_=t_g1[:])
            nc.sync.dma_start(out=g2a_sb[:], in_=t_g2[0:128, :])
            nc.sync.dma_start(out=g2b_sb[:], in_=t_g2[128:200, :])

            # ---------- DRAM staging ----------
            st_dr = {
                "b2": dr.tile([24, 256 * 100], dt.float32, tag="st2", name="st2"),
                "b3": dr.tile([36, 128 * 100], dt.float32, tag="st3", name="st3"),
                "b4": dr.tile([48, 128 * 100], dt.float32, tag="st4", name="st4"),
            }

            # ---------- gathers + stage writes ----------
            for t in range(8):
                for h in range(2):
                    idx_sb = p3.tile([EH, 1], dt.int32, tag="idx")
                    iv = t_idx[:].rearrange("a b -> (a b) ()")
                    nc.sync.dma_start(
                        out=idx_sb[:], in_=iv[(t * 2 + h) * EH : (t * 2 + h + 1) * EH]
                    )
                    gat = p3.tile([EH, D], dt.float32, tag="gat")
                    src = t_ptab if t == 0 else t_remb
                    nc.gpsimd.indirect_dma_start(
                        out=gat[:],
                        out_offset=None,
                        in_=src[:],
                        in_offset=bass.IndirectOffsetOnAxis(ap=idx_sb[:, :1], axis=0),
                    )
                    # windowed stage writes for every (slab,row,epart) fed by tensor t
                    for name, base, n_ci, E_w, tmap in SLABS:
                        st = st_dr[name]
                        nrows = n_ci * 6
                        stv = st[:].rearrange("p (e d) -> p e d", d=100)
                        for (ci, ep), tt in tmap.items():
                            if tt != t:
                                continue
                            e0 = ep * 128
                            for k in range(3):
                                for sl in range(2):
                                    r = ci * 6 + k * 2 + sl
                                    dlo, dhi, slo = WIN[(k, sl)]
                                    w = dhi - dlo
                                    row = h * nrows + r
                                    nc.sync.dma_start(
                                        out=stv[row : row + 1, e0 : e0 + EH, dlo:dhi],
                                        in_=gat[:, slo : slo + w],
                                    )
            # border zero columns in staging
            for name, base, n_ci, E_w, tmap in SLABS:
                st = st_dr[name]
                nrows = n_ci * 6
                stv = st[:].rearrange("p (e d) -> p e d", d=100)
                for h in range(2):
                    for ci in range(n_ci):
                        for ep in range(E_w // 128):
                            e0 = ep * 128
                            r00 = h * nrows + ci * 6 + 0 * 2 + 0
                            r21 = h * nrows + ci * 6 + 2 * 2 + 1
                            nc.sync.dma_start(
                                out=stv[r00 : r00 + 1, e0 : e0 + EH, 0:1], in_=zcol[:]
                            )
                            nc.sync.dma_start(
                                out=stv[r21 : r21 + 1, e0 : e0 + EH, 99:100], in_=zcol[:]
                            )

            def flatten(h):
                for name, base, n_ci, E_w, tmap in SLABS:
                    nrows = n_ci * 6
                    nc.gpsimd.dma_start(
                        out=xs[base : base + nrows, 0 : E_w * 100],
                        in_=st_dr[name][h * nrows : (h + 1) * nrows, :],
                    )

            lhs_of = {"b2": w2s[0:12], "b3": w3s[32:50], "b4": w4s[64:88]}

            # ---------- phase 1: conv + BN1 stats ----------
            for h in range(2):
                flatten(h)
                for name, base, n_ci, E_w, tmap in SLABS:
                    nrows = n_ci * 6
                    xsv = xs[base : base + nrows, 0 : E_w * 100].rearrange(
                        "p (e d) -> p e d", d=100
                    )
                    for ic in range(E_w // 4):
                        yps = pc.tile([100, 400], dt.float32, space="PSUM", tag="yps")
                        nc.tensor.matmul(
                            out=yps[:],
                            lhsT=lhs_of[name],
                            rhs=xsv[:, ic * 4 : (ic + 1) * 4, :],
                            start=True,
                            stop=True,
                        )
                        if name == "b2":
                            gcol = h * 128 + ic  # x1: 0-31, re: 32-63
                        elif name == "b3":
                            gcol = h * 128 + 64 + ic
                        else:
                            gcol = h * 128 + 96 + ic
                        nc.vector.tensor_reduce(
                            out=ssum[:, gcol : gcol + 1],
                            in_=yps[:],
                            axis=mybir.AxisListType.X,
                            op=OP.add,
                        )
                        nc.scalar.activation(
                            out=yps[:],
                            in_=yps[:],
                            func=AF.Square,
                            accum_out=sqacc[:, gcol : gcol + 1],
                        )

            # per-blk stat totals: blk col offsets x1=0 re=32 x2=64 x3=96
            sqv = sqacc[:].rearrange("p (h c) -> p h c", h=2)
            ssv = ssum[:].rearrange("p (h c) -> p h c", h=2)
            for b, off in ((0, 0), (1, 64), (2, 96), (3, 32)):
                nc.vector.tensor_reduce(
                    out=stats[:, b : b + 1],
                    in_=sqv[:, :, off : off + 32],
                    axis=mybir.AxisListType.XY,
                    op=OP.add,
                )
                nc.vector.tensor_reduce(
                    out=stats[:, 4 + b : 5 + b],
                    in_=ssv[:, :, off : off + 32],
                    axis=mybir.AxisListType.XY,
                    op=OP.add,
                )

            # ---------- AllReduce 1 ----------
            ar1i = dr.tile([100, 8], dt.float32, tag="ar1i")
            ar1o = dr.tile([100, 8], dt.float32, tag="ar1o")
            nc.gpsimd.dma_start(out=ar1i[:], in_=stats[:])
            nc.gpsimd.collective_compute(
                "AllReduce",
                OP.add,
                replica_groups=[list(range(NC))],
                ins=[ar1i.opt()],
                outs=[ar1o.opt()],
            )
            nc.gpsimd.dma_start(out=gstats[:], in_=ar1o[:])

            # slab-pair merge: fullstats = pairmat.T @ gstats
            fps = pc.tile([100, 8], dt.float32, space="PSUM", tag="yps")
            nc.tensor.matmul(
                out=fps[:], lhsT=pair_sb[:], rhs=gstats[:], start=True, stop=True
            )
            nc.vector.tensor_copy(out=fs_sb[:], in_=fps[:])

            # affine1: a = g1*rsqrt(var+eps), beta = be1 - a*mu   (per (c,sl), per blk)
            def affine(dst, fs, gbe, invn, npart, scr_pool):
                for b in range(4):
                    mu = scr_pool.tile([npart, 1], dt.float32, tag="afmu")
                    e2 = scr_pool.tile([npart, 1], dt.float32, tag="afe2")
                    tv = scr_pool.tile([npart, 1], dt.float32, tag="aftv")
                    nc.vector.tensor_scalar_mul(out=mu[:], in0=fs[:, 4 + b : 5 + b], scalar1=invn)
                    nc.vector.tensor_scalar_mul(out=e2[:], in0=fs[:, b : b + 1], scalar1=invn)
                    nc.vector.tensor_tensor(out=tv[:], in0=mu[:], in1=mu[:], op=OP.mult)
                    nc.vector.tensor_tensor(out=tv[:], in0=e2[:], in1=tv[:], op=OP.subtract)
                    nc.vector.tensor_scalar_add(out=tv[:], in0=tv[:], scalar1=EPS_BN)
                    nc.scalar.activation(out=tv[:], in_=tv[:], func=AF.Sqrt)
                    nc.vector.reciprocal(out=tv[:], in_=tv[:])
                    nc.vector.tensor_tensor(
                        out=dst[:, 2 * b : 2 * b + 1], in0=tv[:], in1=gbe[:, 0:1], op=OP.mult
                    )
                    nc.vector.tensor_tensor(
                        out=tv[:], in0=dst[:, 2 * b : 2 * b + 1], in1=mu[:], op=OP.mult
                    )
                    nc.vector.tensor_tensor(
                        out=dst[:, 2 * b + 1 : 2 * b + 2], in0=gbe[:, 1:2], in1=tv[:],
                        op=OP.subtract,
                    )

            affine(af1, fs_sb, g1_sb, 1.0 / N1, 100, p3)
            if debug:
                nc.sync.dma_start(out=dbg["d_stats"], in_=stats[:])
                nc.sync.dma_start(out=dbg["d_gstats"], in_=gstats[:])
                nc.sync.dma_start(out=dbg["d_af1"], in_=af1[:])

            # ---------- phase 3: conv again -> relu-affine -> fc ----------
            zpa = pz.tile([128, 1024], dt.float32, space="PSUM", tag="zpa")
            zpb = pz.tile([72, 1024], dt.float32, space="PSUM", tag="zpb")

            chunk_ct = 0
            for h in range(2):
                flatten(h)
                for dq in range(NDQ):
                    fr = p2.tile([100, G * D], dt.float32r, tag="fcwring")
                    nc.gpsimd.dma_start(
                        out=fr[:], in_=t_fcw[:, dq * G : (dq + 1) * G, :]
                    )
                    u2 = p2.tile([100, 4 * G * 128], dt.float32r, tag="u2")
                    u2w = u2[:].rearrange("p (b dl e) -> p b e dl", dl=G, e=128)
                    for name, base, n_ci, E_w, tmap in SLABS:
                        nrows = n_ci * 6
                        xsv = xs[base : base + nrows, 0 : E_w * 100].rearrange(
                            "p (e d) -> p e d", d=100
                        )
                        for ec in range(E_w // 32):
                            yps = pc.tile([100, 320], dt.float32, space="PSUM", tag="yps")
                            nc.tensor.matmul(
                                out=yps[:],
                                lhsT=lhs_of[name],
                                rhs=xsv[:, ec * 32 : (ec + 1) * 32, dq * G : (dq + 1) * G],
                                start=True,
                                stop=True,
                            )
                            if name == "b2":
                                blk = 0 if ec < 4 else 3
                                e0 = (ec % 4) * 32
                            elif name == "b3":
                                blk, e0 = 1, ec * 32
                            else:
                                blk, e0 = 2, ec * 32
                            ypv = yps[:].rearrange("p (e dl) -> p e dl", dl=G)
                            dst = u2w[:, blk, e0 : e0 + 32, :]
                            sc = af1[:, 2 * blk : 2 * blk + 1]
                            bi = af1[:, 2 * blk + 1 : 2 * blk + 2]
                            if chunk_ct % 3 == 0:
                                tmp = p2.tile([100, 320], dt.float32, tag="dvetmp")
                                tpv = tmp[:].rearrange("p (e dl) -> p e dl", dl=G)
                                nc.vector.tensor_scalar(
                                    out=tpv, in0=ypv, scalar1=sc, scalar2=bi,
                                    op0=OP.mult, op1=OP.add,
                                )
                                nc.vector.tensor_scalar_max(out=dst, in0=tpv, scalar1=0.0)
                            else:
                                nc.scalar.activation(
                                    out=dst, in_=ypv, func=AF.Relu, bias=bi, scale=sc
                                )
                            chunk_ct += 1
                    if debug and h == 0 and dq == 0:
                        u2f = u2[:].bitcast(dt.float32)
                        nc.sync.dma_start(out=dbg["d_u2"], in_=u2f)
                        nc.sync.dma_start(out=dbg["d_xs"], in_=xs[:].bitcast(dt.float32))
                    u2v = u2[:].rearrange("p (b dl e) -> p dl b e", dl=G, e=128)
                    for dl in range(G):
                        first = dq == 0 and dl == 0
                        last = dq == NDQ - 1 and dl == G - 1
                        nc.tensor.matmul(
                            out=zpa[:, h * 512 : (h + 1) * 512],
                            lhsT=fr[:, dl * D : dl * D + 128],
                            rhs=u2v[:, dl],
                            start=first,
                            stop=last,
                        )
                        nc.tensor.matmul(
                            out=zpb[:, h * 512 : (h + 1) * 512],
                            lhsT=fr[:, dl * D + 128 : (dl + 1) * D],
                            rhs=u2v[:, dl],
                            start=first,
                            stop=last,
                        )

            # ---------- BN2 stats ----------
            for jt, (zp, stt, npart) in enumerate(
                ((zpa, st2a, 128), (zpb, st2b, 72))
            ):
                zv = zp[:].rearrange("p (h b e) -> p h b e", h=2, b=4)
                for b in range(4):
                    nc.vector.tensor_reduce(
                        out=stt[:, 4 + b : 5 + b],
                        in_=zv[:, :, b, :],
                        axis=mybir.AxisListType.XY,
                        op=OP.add,
                    )
                    junk = p2.tile([npart, 256], dt.float32, tag=f"junk{jt}")
                    jv = junk[:].rearrange("p (h e) -> p h e", h=2)
                    nc.scalar.activation(
                        out=jv, in_=zv[:, :, b, :], func=AF.Square,
                        accum_out=stt[:, b : b + 1],
                    )

            if debug:
                ztmp = p1.tile([128, 1024], dt.float32, tag="ztmp")
                nc.vector.tensor_copy(out=ztmp[:], in_=zpa[:])
                nc.sync.dma_start(out=dbg["d_z"], in_=ztmp[:])
                ztmpb = p1.tile([72, 1024], dt.float32, tag="ztmpb")
                nc.vector.tensor_copy(out=ztmpb[:], in_=zpb[:])
                nc.sync.dma_start(out=dbg["d_zb"], in_=ztmpb[:])
                nc.sync.dma_start(out=dbg["d_st2a"], in_=st2a[:])
            ar2i = dr.tile([D, 8], dt.float32, tag="ar2i")
            ar2o = dr.tile([D, 8], dt.float32, tag="ar2o")
            nc.gpsimd.dma_start(out=ar2i[0:128, :], in_=st2a[:])
            nc.gpsimd.dma_start(out=ar2i[128:200, :], in_=st2b[:])
            nc.gpsimd.collective_compute(
                "AllReduce",
                OP.add,
                replica_groups=[list(range(NC))],
                ins=[ar2i.opt()],
                outs=[ar2o.opt()],
            )
            nc.gpsimd.dma_start(out=gst2a[:], in_=ar2o[0:128, :])
            nc.gpsimd.dma_start(out=gst2b[:], in_=ar2o[128:200, :])

            affine(af2a, gst2a, g2a_sb, 1.0 / N2, 128, p3)
            if debug:
                nc.sync.dma_start(out=dbg["d_af2a"], in_=af2a[:])
            affine(af2b, gst2b, g2b_sb, 1.0 / N2, 72, p3)

            # normalize + relu -> xnorm
            for zp, xn, af in ((zpa, xna, af2a), (zpb, xnb, af2b)):
                for b in range(4):
                    for h in range(2):
                        cl = h * 512 + b * 128
                        nc.scalar.activation(
                            out=xn[:, cl : cl + 128],
                            in_=zp[:, cl : cl + 128],
                            func=AF.Relu,
                            scale=af[:, 2 * b : 2 * b + 1],
                            bias=af[:, 2 * b + 1 : 2 * b + 2],
                        )

            if debug:
                nc.sync.dma_start(out=dbg["d_xna"], in_=xna[:])
                nc.sync.dma_start(out=dbg["d_xnb"], in_=xnb[:])
            # ---------- cos + scatter ----------
            for h in range(2):
                ds = p3.tile([128, 8], dt.float32, tag="ds")
                for col in range(7):
                    # product pairs: 0-2 = x_b . re, 3-5 = x_b . x_b, 6 = re . re
                    dps = pc.tile([128, 1], dt.float32, space="PSUM", tag="dps")
                    for jt, (xn, npart) in enumerate(((xna, 128), (xnb, 72))):
                        re_sl = xn[:, h * 512 + 3 * 128 : h * 512 + 4 * 128]
                        if col < 3:
                            a_ap = xn[:, h * 512 + col * 128 : h * 512 + (col + 1) * 128]
                            b_ap = re_sl
                        elif col < 6:
                            a_ap = xn[:, h * 512 + (col - 3) * 128 : h * 512 + (col - 2) * 128]
                            b_ap = a_ap
                        else:
                            a_ap = re_sl
                            b_ap = re_sl
                        prod = p3.tile([npart, 128], dt.float32, tag=f"prod{jt}")
                        nc.vector.tensor_tensor(out=prod[:], in0=a_ap, in1=b_ap, op=OP.mult)
                        nc.tensor.matmul(
                            out=dps[:],
                            lhsT=prod[:],
                            rhs=ones[0:npart, :],
                            start=(jt == 0),
                            stop=(jt == 1),
                        )
                    nc.vector.tensor_copy(out=ds[:, col : col + 1], in_=dps[:])
                if debug and h == 0:
                    nc.sync.dma_start(out=dbg["d_ds"], in_=ds[:, :])
                csb = p3.tile([128, 1], dt.float32, tag="csb")
                for b in range(3):
                    m = p3.tile([128, 1], dt.float32, tag="cosm")
                    nc.vector.tensor_tensor(
                        out=m[:], in0=ds[:, 3 + b : 4 + b], in1=ds[:, 6:7], op=OP.mult
                    )
                    nc.scalar.activation(out=m[:], in_=m[:], func=AF.Sqrt)
                    nc.vector.tensor_scalar_max(out=m[:], in0=m[:], scalar1=EPS_COS)
                    nc.vector.reciprocal(out=m[:], in_=m[:])
                    nc.vector.tensor_tensor(out=m[:], in0=m[:], in1=ds[:, b : b + 1], op=OP.mult)
                    if b == 0:
                        nc.vector.tensor_copy(out=csb[:], in_=m[:])
                    else:
                        nc.vector.tensor_tensor(out=csb[:], in0=csb[:], in1=m[:], op=OP.add)
                if debug:
                    nc.sync.dma_start(out=dbg["d_csb"][:, h : h + 1], in_=csb[:])
                si = p3.tile([128, 1], dt.int32, tag="si")
                siv = t_sidx[:].rearrange("a b -> (a b) ()")
                nc.sync.dma_start(out=si[:], in_=siv[h * EH : (h + 1) * EH])
                nc.gpsimd.indirect_dma_start(
                    out=t_out[:].rearrange("a b -> (a b) ()"),
                    out_offset=bass.IndirectOffsetOnAxis(ap=si[:, :1], axis=0),
                    in_=csb[:],
                    in_offset=None,
                )

    nc.compile()
    return nc


def _prep_core(core, pre_emb, r_emb, W2, W3, W4, fc_w, g1, be1, g2, be2,
               src_ids, edge_type, p1, p2, p3, o):
    lo, hi = core * ESH, (core + 1) * ESH
    s_ids = np.asarray(src_ids[lo:hi], dtype=np.int64)
    uniq, inv = np.unique(s_ids, return_inverse=True)
    ptab = np.zeros((ESH, D), np.float32)
    ptab[: len(uniq)] = pre_emb[uniq]
    idxs = [
        inv.astype(np.int32),
        np.asarray(p1[lo:hi], np.int32),
        np.asarray(edge_type[lo:hi], np.int32),
        np.asarray(p2[lo:hi, 0], np.int32),
        np.asarray(p2[lo:hi, 1], np.int32),
        np.asarray(p3[lo:hi, 0], np.int32),
        np.asarray(p3[lo:hi, 1], np.int32),
        np.asarray(p3[lo:hi, 2], np.int32),
    ]
    idx_all = np.zeros((16, EH), np.int32)
    for t in range(8):
        idx_all[2 * t] = idxs[t][:EH]
        idx_all[2 * t + 1] = idxs[t][EH:]

    def wflat(W, n_ci):
        out = np.zeros((n_ci * 6, 100), np.float32)
        for ci in range(n_ci):
            for k in range(3):
                for sl in range(2):
                    out[ci * 6 + k * 2 + sl, sl * 50 : (sl + 1) * 50] = W[:, ci, k]
        return out

    fcw = (
        fc_w.reshape(D, C, 2, 100).transpose(2, 1, 3, 0).reshape(100, 100, D)
    ).astype(np.float32).copy()
    pairmat = (np.arange(100)[:, None] % 50 == np.arange(100)[None, :] % 50).astype(
        np.float32
    )
    g1be1 = np.stack([np.tile(g1, 2), np.tile(be1, 2)], axis=1).astype(np.float32)
    g2be2 = np.stack([g2, be2], axis=1).astype(np.float32)
    o_sh = np.asarray(o[lo:hi], np.int64)
    scat = (np.arange(ESH, dtype=np.int64) * NN + o_sh).astype(np.int32)
    return {
        "ptab": ptab,
        "remb": np.asarray(r_emb, np.float32),
        "idx_all": idx_all,
        "w2f": wflat(np.asarray(W2, np.float32), 2),
        "w3f": wflat(np.asarray(W3, np.float32), 3),
        "w4f": wflat(np.asarray(W4, np.float32), 4),
        "fcw": fcw,
        "pairmat": pairmat,
        "g1be1": g1be1,
        "g2be2": g2be2,
        "scatidx": scat.reshape(2, EH),
    }


def kernel(pre_emb, r_emb, W2, b2, W3, b3, W4, b4, fc_w, fc_b,
           g1, be1, g2, be2, src_ids, edge_type, p1, p2, p3, o,
           _trace=False, _debug=False):
    """Full-input / full-output entry point. b2/b3/b4/fc_b cancel inside the
    training-mode BatchNorms and are unused."""
    from concourse.bass_utils import run_bass_kernel_spmd

    key = "ncd" if _debug else "nc"
    if key not in _CACHE:
        _CACHE[key] = _build(debug=_debug)
    nc = _CACHE[key]

    args = (pre_emb, r_emb, W2, W3, W4, fc_w, g1, be1, g2, be2,
            src_ids, edge_type, p1, p2, p3, o)
    in_maps = [_prep_core(c, *[np.asarray(a) for a in args]) for c in range(NC)]
    res = run_bass_kernel_spmd(
        nc, in_maps, core_ids=list(range(NC)), trace=_trace
    )
    _CACHE["last_results"] = res
    out = np.concatenate([res.results[c]["out"] for c in range(NC)], axis=0)
    return out.astype(np.float32)
